# revision 38
# baseline (speedup 1.0000x reference)
"""LlamaAttention (B=1, S=2048, D=2048, H=16, KV=4) on 8 TRN2 NeuronCores.

Tensor-parallel over heads: core c owns q-heads [2c, 2c+1] and kv-head c//2.
Each core computes partial = attn_out_c @ Wo[:, c-slice].T over the full
sequence; the all-reduce after o_proj happens on the host (sum of partials).

Layout strategy: everything on-chip lives feature-on-partitions ("transposed"):
  hsT [d, s], qT/kT/vT [hd, s], attn_outT [hd, s].  The host pre-transposes
hidden_states and weights into partition-major [128, N] bf16 arrays so every
DMA is contiguous; rope tables (bf16 cos / sign-adjusted sin) and the causal
diagonal mask block are precomputed on host.

Schedule (all matmuls bf16: fast weight load, half HBM):
 - DMA prologue orders the first hs tile + first weight chunks ahead of the
   bulk so the PE starts within a few microseconds.
 - QKV projects in four 512-column PSUM quarters (4 banks), leaving 4 banks
   for attention score chunks: the first five attention units' score/softmax
   chunks interleave into the back half of the projection matmul stream, so
   softmax runs under projection and the PE never idles at the phase seam.
 - P^T and natural-layout V come from xbar DMA transposes issued per tile
   right after normalization (no PE transposes, no PSUM drain copies); P@V
   reads the transposed tiles straight from SBUF through a strided AP.
 - Remaining units' score generation is pumped one chunk at a time between
   P@V accumulation steps, and each group's o_proj tiles are split around
   unit seams, so the in-order PE queue always has matmul work while
   exp (scalar) and normalization (vector) catch up.
 - softmax: exp with accum_out row sums (no running max: scores are O(6)
   sigma so fp32 exp cannot overflow); P normalized in sbuf by 1/l.
 - PSUM drains: rope uses one scalar cast then all-bf16 DVE ops (the
   rotate-half is a partition-shifted DVE copy); o_proj casts alternate
   scalar/vector.  Output partials are bf16, host all-reduces in f32.
"""
import math
import numpy as np

S = 2048
D = 2048
HD = 128
H = 16
KV = 4
NCORES = 8
NT = S // 128          # 16 sequence tiles
DTC = D // 128         # 16 feature chunks
QH = H // NCORES       # 2 q-heads per core
ROPE_BASE = 10000.0
SCALE = 1.0 / math.sqrt(HD)
NEG = -1.0e9

_CACHE = {}


def _rope(nc, pool, dst, src_ps, cos_sb, sin_sb, cols, BF16, ALU):
    """dst[:, cols] = src*cos + rotate_half(src)*sin  (src: psum [128, w]).

    One scalar drain (psum->bf16 sbuf), then all-bf16 SBUF vector ops which
    run in the DVE's fast 2x/4x modes; the rotate-half is a partition-shifted
    DVE copy (legal on TRN2).
    """
    w = cols.stop - cols.start
    raw = pool.tile([128, w], BF16, tag="roperaw")
    rot = pool.tile([128, w], BF16, tag="roperot")
    t1 = pool.tile([128, w], BF16, tag="ropet1")
    nc.scalar.copy(out=raw, in_=src_ps)
    nc.vector.tensor_copy(out=rot[0:64, :], in_=raw[64:128, :])
    nc.vector.tensor_copy(out=rot[64:128, :], in_=raw[0:64, :])
    nc.vector.tensor_tensor(out=t1, in0=raw, in1=cos_sb[:, cols], op=ALU.mult)
    nc.vector.tensor_tensor(out=rot, in0=rot, in1=sin_sb[:, cols], op=ALU.mult)
    nc.vector.tensor_tensor(out=dst[:, cols], in0=t1, in1=rot, op=ALU.add)


def build_nc():
    import concourse.bacc as bacc
    import concourse.tile as tile
    from concourse import mybir

    F32 = mybir.dt.float32
    BF16 = mybir.dt.bfloat16
    AF = mybir.ActivationFunctionType
    ALU = mybir.AluOpType

    nc = bacc.Bacc("TRN2", target_bir_lowering=False, debug=False)
    hs_d = nc.dram_tensor("hs", [128, DTC * S], BF16, kind="ExternalInput").ap()
    wq_d = nc.dram_tensor("wq", [128, DTC * QH * 128], BF16, kind="ExternalInput").ap()
    wk_d = nc.dram_tensor("wk", [128, DTC * 128], BF16, kind="ExternalInput").ap()
    wv_d = nc.dram_tensor("wv", [128, DTC * 128], BF16, kind="ExternalInput").ap()
    wo_d = nc.dram_tensor("wo", [128, QH * D], BF16, kind="ExternalInput").ap()
    cos_d = nc.dram_tensor("cos", [128, S], BF16, kind="ExternalInput").ap()
    sin_d = nc.dram_tensor("sin", [128, S], BF16, kind="ExternalInput").ap()
    tri_d = nc.dram_tensor("tri", [128, 128], F32, kind="ExternalInput").ap()
    out_d = nc.dram_tensor("out", [128, NT * D], BF16, kind="ExternalOutput").ap()

    hs3 = hs_d.rearrange("p (t s) -> p t s", t=DTC)
    out3 = out_d.rearrange("p (t d) -> p t d", t=NT)

    HALF = S // 2
    QTR = S // 4

    with tile.TileContext(nc) as tc:
        with tc.tile_pool(name="consts", bufs=1) as consts, \
             tc.tile_pool(name="persist", bufs=1) as persist, \
             tc.tile_pool(name="stats", bufs=1) as stats, \
             tc.tile_pool(name="pp", bufs=8) as pp, \
             tc.tile_pool(name="ptt", bufs=3) as ptt, \
             tc.tile_pool(name="osb", bufs=3) as osb, \
             tc.tile_pool(name="sps", bufs=2, space="PSUM") as sps:
            tri_sb = consts.tile([128, 128], F32)
            cos_sb = consts.tile([128, S], BF16)
            sin_sb = consts.tile([128, S], BF16)
            wq_sb = consts.tile([128, DTC, QH * 128], BF16)
            wk_sb = consts.tile([128, DTC, 128], BF16)
            wv_sb = consts.tile([128, DTC, 128], BF16)
            wo_sb = consts.tile([128, QH, D], BF16)

            qrot = [persist.tile([128, S], BF16, tag=f"qrot{h}", name=f"qrot{h}") for h in range(QH)]
            krot = persist.tile([128, S], BF16, tag="krot")
            vbf = persist.tile([128, S], BF16, tag="vbf")
            vnat = persist.tile([128, NT * 128], BF16, tag="vnat")
            vnat3 = vnat.rearrange("p (t f) -> p t f", t=NT)
            aout = [persist.tile([128, S], BF16, tag=f"aout{h}", name=f"aout{h}") for h in range(QH)]
            l_sb = stats.tile([128, QH * NT], F32, tag="l")
            linv_sb = stats.tile([128, QH * NT], F32, tag="linv")
            lpart = stats.tile([128, QH * NT * 2], F32, tag="lpart")

            units = [(g, h) for g in range(NT // 4) for h in range(QH)]

            def scores_gen(u):
                """QK chunks + mask + exp + normalize + P^T xbar for unit u.

                Yields after each score chunk so the caller can interleave
                other engine work; finishes by issuing the transpose DMAs
                into ptall and returning it via StopIteration.value.
                """
                g, h = units[u]
                for ii in range(4):
                    i = 4 * g + ii
                    W = (i + 1) * 128
                    p_i = pp.tile([128, S], BF16, tag="p", name=f"p{u}_{ii}")
                    col = h * NT + i
                    nch = (W + 1023) // 1024
                    for c in range(nch):
                        c0 = 1024 * c
                        ce = min(c0 + 1024, W)
                        s_ch = sps.tile([128, 1024], F32, tag="s")
                        for m0 in range(c0, ce, 512):
                            m1 = min(m0 + 512, ce)
                            nc.tensor.matmul(s_ch[:, m0 - c0:m1 - c0],
                                             qrot[h][:, i * 128:(i + 1) * 128],
                                             krot[:, m0:m1], start=True, stop=True)
                        if ce == W:   # diagonal block lives in this chunk
                            nc.vector.tensor_tensor(
                                out=s_ch[:, W - 128 - c0:W - c0],
                                in0=s_ch[:, W - 128 - c0:W - c0],
                                in1=tri_sb, op=ALU.add)
                        nc.scalar.activation(out=p_i[:, c0:ce], in_=s_ch[:, 0:ce - c0],
                                             func=AF.Exp, scale=SCALE,
                                             accum_out=lpart[:, col * 2 + c:col * 2 + c + 1])
                        yield
                    if nch > 1:
                        nc.vector.tensor_reduce(out=l_sb[:, col:col + 1],
                                                in_=lpart[:, col * 2:col * 2 + nch],
                                                axis=mybir.AxisListType.X, op=ALU.add)
                        nc.vector.reciprocal(out=linv_sb[:, col:col + 1],
                                             in_=l_sb[:, col:col + 1])
                    else:
                        nc.vector.reciprocal(out=linv_sb[:, col:col + 1],
                                             in_=lpart[:, col * 2:col * 2 + 1])
                    nc.vector.tensor_scalar_mul(p_i[:, 0:W], p_i[:, 0:W],
                                                linv_sb[:, col:col + 1])
                    if ii == 0:
                        ptall = ptt.tile([128, 4, NT, 128], BF16, tag="ptall",
                                         name=f"ptall{u}")
                    nb = W // 128
                    if nb > 8:   # split so P@V can start on the first half
                        nc.sync.dma_start_transpose(
                            out=ptall[:, ii, 0:8, :], in_=p_i[:, 0:1024])
                        nc.sync.dma_start_transpose(
                            out=ptall[:, ii, 8:nb, :], in_=p_i[:, 1024:W])
                    else:
                        nc.sync.dma_start_transpose(
                            out=ptall[:, ii, 0:nb, :], in_=p_i[:, 0:W])
                return ptall

            def run_gen(gen):
                while True:
                    try:
                        next(gen)
                    except StopIteration as e:
                        return e.value

            # ---------------- QKV in quarters + early attention ------------
            hst_tiles = {}

            with tc.tile_pool(name="hsp", bufs=8) as hsp, \
                 tc.tile_pool(name="ropet", bufs=1) as ropet, \
                 tc.tile_pool(name="qkvps", bufs=1, space="PSUM") as qkvps:

                def load_hst(sh, j):
                    t = hsp.tile([128, 2, HALF], BF16, tag="hst", name=f"hst{sh}_{j}")
                    nc.sync.dma_start(
                        out=t, in_=hs3[:, 2 * j:2 * j + 2, sh * HALF:(sh + 1) * HALF])
                    hst_tiles[(sh, j)] = t
                    return t

                # DMA prologue: tiny mask, first weight chunks + hs tiles (so
                # the first matmuls start ~4us in), bulk weights, rest of hs
                # half 0, rope tables, wo (needed later).
                wq3 = wq_d.rearrange("p (t m) -> p t m", t=DTC)
                wk3 = wk_d.rearrange("p (t m) -> p t m", t=DTC)
                wv3 = wv_d.rearrange("p (t m) -> p t m", t=DTC)
                load_hst(0, 0)
                nc.sync.dma_start(out=wq_sb[:, 0:2, :], in_=wq3[:, 0:2, :])
                nc.sync.dma_start(out=wk_sb[:, 0:2, :], in_=wk3[:, 0:2, :])
                nc.sync.dma_start(out=wv_sb[:, 0:2, :], in_=wv3[:, 0:2, :])
                load_hst(0, 1)
                nc.sync.dma_start(out=tri_sb, in_=tri_d)
                nc.sync.dma_start(out=wq_sb[:, 2:DTC, :], in_=wq3[:, 2:DTC, :])
                nc.sync.dma_start(out=wk_sb[:, 2:DTC, :], in_=wk3[:, 2:DTC, :])
                nc.sync.dma_start(out=wv_sb[:, 2:DTC, :], in_=wv3[:, 2:DTC, :])
                for j in range(2, 8):
                    load_hst(0, j)
                nc.sync.dma_start(out=cos_sb, in_=cos_d)
                nc.sync.dma_start(out=sin_sb, in_=sin_d)
                nc.sync.dma_start(out=wo_sb, in_=wo_d.rearrange("p (h m) -> p h m", h=QH))

                early = []
                pending = []
                for qtr in range(4):
                    sh, qq = divmod(qtr, 2)
                    cols = slice(qtr * QTR, (qtr + 1) * QTR)
                    if qtr == 1:
                        for j in range(8):   # prefetch half 1 as slots free up
                            load_hst(1, j)
                    if qtr == 2:
                        pending = [scores_gen(0), scores_gen(1)]
                    if qtr == 3:
                        pending += [scores_gen(2), scores_gen(3), scores_gen(4)]
                    pq = [qkvps.tile([128, QTR], F32, tag=f"pq{m}", name=f"pq{m}") for m in range(QH)]
                    pk = qkvps.tile([128, QTR], F32, tag="pk")
                    pv = qkvps.tile([128, QTR], F32, tag="pv")
                    for j in range(DTC // 2):
                        hst = hst_tiles[(sh, j)]
                        for t2 in range(2):
                            dt = 2 * j + t2
                            st = dt == 0
                            sp = dt == DTC - 1
                            wlist = ([(wq_sb[:, dt, m * 128:(m + 1) * 128], pq[m]) for m in range(QH)]
                                     + [(wk_sb[:, dt, :], pk), (wv_sb[:, dt, :], pv)])
                            for w_ap, dst in wlist:
                                nc.tensor.matmul(dst, w_ap, hst[:, t2, qq * QTR:(qq + 1) * QTR],
                                                 start=st, stop=sp)
                        for _ in range(2 if qtr == 3 else 1):
                            if pending:
                                try:
                                    next(pending[0])
                                except StopIteration as e:
                                    early.append(e.value)
                                    pending.pop(0)
                    for m in range(QH):
                        _rope(nc, ropet, qrot[m], pq[m], cos_sb, sin_sb, cols, BF16, ALU)
                    _rope(nc, ropet, krot, pk, cos_sb, sin_sb, cols, BF16, ALU)
                    nc.scalar.copy(out=vbf[:, cols], in_=pv)
                    nc.sync.dma_start_transpose(
                        out=vnat3[:, 4 * qtr:4 * qtr + 4, :], in_=vbf[:, cols])


            # ---------------- attention tail + fused o_proj ----------------
            with tc.tile_pool(name="pvps", bufs=2, space="PSUM") as pvps, \
                 tc.tile_pool(name="pops", bufs=2, space="PSUM") as pops:

                deferred = []

                def oproj_tile(t, g):
                    o_sb = osb.tile([128, D], BF16, tag="osb")
                    for n in range(D // 512):
                        po = pops.tile([128, 512], F32, tag="po", name=f"po{t}_{n}")
                        for hh in range(QH):
                            nc.tensor.matmul(po, aout[hh][:, t * 128:(t + 1) * 128],
                                             wo_sb[:, hh, n * 512:(n + 1) * 512],
                                             start=(hh == 0), stop=(hh == QH - 1))
                        on_scalar = (n % 2 == 0) if g == 3 else (n == 0)
                        if on_scalar:
                            nc.scalar.copy(out=o_sb[:, n * 512:(n + 1) * 512], in_=po)
                        else:
                            nc.vector.tensor_copy(out=o_sb[:, n * 512:(n + 1) * 512], in_=po)
                    nc.sync.dma_start(out=out3[:, t, :], in_=o_sb)

                def stage_pv(u, ptall, pump):
                    """P@V accumulation + aout; o_proj split around unit seams."""
                    g, h = units[u]
                    jmax = 4 * g + 3
                    pv_ps = pvps.tile([128, 512], F32, tag="pv")
                    for j in range(jmax + 1):
                        ii_lo = max(0, j - 4 * g)
                        nc.tensor.matmul(pv_ps[:, ii_lo * 128:512],
                                         vnat[:, j * 128:(j + 1) * 128],
                                         ptall[:, ii_lo:4, j, :],
                                         start=(j == 0), stop=(j == jmax))
                        pump()
                        if deferred and j % 3 == 2:   # PE filler between PV steps
                            oproj_tile(*deferred.pop(0))
                    while deferred:
                        oproj_tile(*deferred.pop(0))
                    nc.scalar.copy(out=aout[h][:, g * 512:(g + 1) * 512],
                                   in_=pv_ps)
                    if h == QH - 1:   # both heads done: 1 tile now, 3 deferred
                        oproj_tile(4 * g, g)
                        deferred.extend([(4 * g + 1, g), (4 * g + 2, g),
                                         (4 * g + 3, g)])

                # Pump the next units' score generation between PV matmuls so
                # the in-order PE queue always has matmul work while softmax
                # (scalar/vector) of later units catches up.
                ptalls = dict(enumerate(early))
                live = {}
                nxt = len(early)
                for gen in pending:
                    live[nxt] = gen
                    nxt += 1
                for u in range(nxt, len(units)):
                    live[u] = scores_gen(u)

                def pump():
                    for u in sorted(live):
                        try:
                            next(live[u])
                        except StopIteration as e:
                            ptalls[u] = e.value
                            del live[u]
                        return

                for u in range(len(units)):
                    while u not in ptalls:   # finish its scores if still pending
                        pump()
                    stage_pv(u, ptalls.pop(u), pump)
                while deferred:
                    oproj_tile(*deferred.pop(0))

    nc.compile()
    return nc


def _pm(x):
    """[n*128, M] row-major -> partition-major [128, n*M]."""
    n = x.shape[0] // 128
    return np.ascontiguousarray(
        x.reshape(n, 128, x.shape[1]).transpose(1, 0, 2).reshape(128, -1))


def prep_in_maps(hidden_states, position_ids, Wq, Wk, Wv, Wo):
    import ml_dtypes
    BF = ml_dtypes.bfloat16
    hs = np.asarray(hidden_states, dtype=np.float32).reshape(S, D)
    hsT_pm = _pm(np.ascontiguousarray(hs.T)).astype(BF)             # [128, DTC*S]

    pos = np.asarray(position_ids).reshape(S).astype(np.float32)
    inv = (ROPE_BASE ** (-np.arange(0, HD, 2, dtype=np.float32) / HD))  # [64]
    ang = np.concatenate([pos[None, :] * inv[:, None]] * 2, axis=0)     # [128, S]
    cos_t = np.cos(ang).astype(BF)
    sin_t = np.sin(ang).astype(np.float32)
    sin_signed = np.concatenate([-sin_t[:64], sin_t[64:]], axis=0).astype(BF)

    q_idx = np.arange(128)[:, None]
    k_idx = np.arange(128)[None, :]
    tri = np.where(k_idx <= q_idx, 0.0, NEG).astype(np.float32)

    Wq = np.asarray(Wq, np.float32)
    Wk = np.asarray(Wk, np.float32)
    Wv = np.asarray(Wv, np.float32)
    Wo = np.asarray(Wo, np.float32)

    in_maps = []
    for c in range(NCORES):
        g = (c * QH) // (H // KV)          # kv head owned by this core
        wq_c = Wq[c * QH * 128:(c + 1) * QH * 128]      # [256, D]
        wk_c = Wk[g * 128:(g + 1) * 128]                # [128, D]
        wv_c = Wv[g * 128:(g + 1) * 128]                # [128, D]
        wo_c = Wo[:, c * QH * 128:(c + 1) * QH * 128]   # [D, 256]
        in_maps.append({
            "hs": hsT_pm,
            "wq": _pm(np.ascontiguousarray(wq_c.T)).astype(BF),
            "wk": _pm(np.ascontiguousarray(wk_c.T)).astype(BF),
            "wv": _pm(np.ascontiguousarray(wv_c.T)).astype(BF),
            "wo": _pm(np.ascontiguousarray(wo_c.T)).astype(BF),
            "cos": cos_t,
            "sin": sin_signed,
            "tri": tri,
        })
    return in_maps


def combine_outputs(results):
    total = np.zeros((S, D), np.float32)
    for r in results:
        o = np.asarray(r["out"], np.float32)
        total += o.reshape(128, NT, D).transpose(1, 0, 2).reshape(S, D)
    return total[None]


def kernel(hidden_states, attention_mask, position_ids, Wq, Wk, Wv, Wo):
    from concourse import bass_utils
    if "nc" not in _CACHE:
        _CACHE["nc"] = build_nc()
    nc = _CACHE["nc"]
    in_maps = prep_in_maps(hidden_states, position_ids, Wq, Wk, Wv, Wo)
    res = bass_utils.run_bass_kernel_spmd(nc, in_maps, core_ids=list(range(NCORES)))
    return combine_outputs(res.results)


# revision 39
# speedup vs baseline: 1.0708x; 1.0708x over previous
"""LlamaAttention (B=1, S=2048, D=2048, H=16, KV=4) on 8 TRN2 NeuronCores.

Tensor-parallel over heads: core c owns q-heads [2c, 2c+1] and kv-head c//2.
Each core computes partial = attn_out_c @ Wo[:, c-slice].T over the full
sequence; the all-reduce after o_proj happens on the host (sum of partials).

Layout strategy: everything on-chip lives feature-on-partitions ("transposed"):
  hsT [d, s], qT/kT/vT [hd, s], attn_outT [hd, s].  The host pre-transposes
hidden_states and weights into partition-major [128, N] bf16 arrays so every
DMA is contiguous; rope tables (bf16 cos / sign-adjusted sin) and the causal
diagonal mask block are precomputed on host.

Schedule (all matmuls bf16: fast weight load, half HBM):
 - DMA prologue orders the first hs tile + first weight chunks ahead of the
   bulk so the PE starts within a few microseconds.
 - QKV projects in four 512-column PSUM quarters (4 banks), leaving 4 banks
   for attention score chunks: the first five attention units' score/softmax
   chunks interleave into the back half of the projection matmul stream, so
   softmax runs under projection and the PE never idles at the phase seam.
 - P^T and natural-layout V come from xbar DMA transposes issued per tile
   right after normalization (no PE transposes, no PSUM drain copies); P@V
   reads the transposed tiles straight from SBUF through a strided AP.
 - Remaining units' score generation is pumped one chunk at a time between
   P@V accumulation steps, and each group's o_proj tiles are split around
   unit seams, so the in-order PE queue always has matmul work while
   exp (scalar) and normalization (vector) catch up.
 - softmax: exp with accum_out row sums (no running max: scores are O(6)
   sigma so fp32 exp cannot overflow); P normalized in sbuf by 1/l.
 - PSUM drains: rope uses one scalar cast then all-bf16 DVE ops (the
   rotate-half is a partition-shifted DVE copy); o_proj casts alternate
   scalar/vector.  Output partials are bf16, host all-reduces in f32.
"""
import math
import numpy as np

S = 2048
D = 2048
HD = 128
H = 16
KV = 4
NCORES = 8
NT = S // 128          # 16 sequence tiles
DTC = D // 128         # 16 feature chunks
QH = H // NCORES       # 2 q-heads per core
ROPE_BASE = 10000.0
SCALE = 1.0 / math.sqrt(HD)
NEG = -1.0e9

_CACHE = {}


def _rope(nc, pool, dst, src_ps, cos_sb, sin_sb, cols, BF16, ALU):
    """dst[:, cols] = src*cos + rotate_half(src)*sin  (src: psum [128, w]).

    One scalar drain (psum->bf16 sbuf), then all-bf16 SBUF vector ops which
    run in the DVE's fast 2x/4x modes; the rotate-half is a partition-shifted
    DVE copy (legal on TRN2).
    """
    w = cols.stop - cols.start
    raw = pool.tile([128, w], BF16, tag="roperaw")
    rot = pool.tile([128, w], BF16, tag="roperot")
    t1 = pool.tile([128, w], BF16, tag="ropet1")
    nc.scalar.copy(out=raw, in_=src_ps)
    nc.vector.tensor_copy(out=rot[0:64, :], in_=raw[64:128, :])
    nc.vector.tensor_copy(out=rot[64:128, :], in_=raw[0:64, :])
    nc.vector.tensor_tensor(out=t1, in0=raw, in1=cos_sb[:, cols], op=ALU.mult)
    nc.vector.tensor_tensor(out=rot, in0=rot, in1=sin_sb[:, cols], op=ALU.mult)
    nc.vector.tensor_tensor(out=dst[:, cols], in0=t1, in1=rot, op=ALU.add)


def build_nc():
    import concourse.bacc as bacc
    import concourse.tile as tile
    from concourse import mybir

    F32 = mybir.dt.float32
    BF16 = mybir.dt.bfloat16
    AF = mybir.ActivationFunctionType
    ALU = mybir.AluOpType

    nc = bacc.Bacc("TRN2", target_bir_lowering=False, debug=False)
    hs_d = nc.dram_tensor("hs", [128, DTC * S], BF16, kind="ExternalInput").ap()
    wq_d = nc.dram_tensor("wq", [128, DTC * QH * 128], BF16, kind="ExternalInput").ap()
    wk_d = nc.dram_tensor("wk", [128, DTC * 128], BF16, kind="ExternalInput").ap()
    wv_d = nc.dram_tensor("wv", [128, DTC * 128], BF16, kind="ExternalInput").ap()
    wo_d = nc.dram_tensor("wo", [128, QH * D], BF16, kind="ExternalInput").ap()
    cos_d = nc.dram_tensor("cos", [128, S], BF16, kind="ExternalInput").ap()
    sin_d = nc.dram_tensor("sin", [128, S], BF16, kind="ExternalInput").ap()
    tri_d = nc.dram_tensor("tri", [128, 128], F32, kind="ExternalInput").ap()
    out_d = nc.dram_tensor("out", [128, NT * D], BF16, kind="ExternalOutput").ap()

    hs3 = hs_d.rearrange("p (t s) -> p t s", t=DTC)
    out3 = out_d.rearrange("p (t d) -> p t d", t=NT)

    HALF = S // 2
    QTR = S // 4

    with tile.TileContext(nc) as tc:
        with tc.tile_pool(name="consts", bufs=1) as consts, \
             tc.tile_pool(name="persist", bufs=1) as persist, \
             tc.tile_pool(name="stats", bufs=1) as stats, \
             tc.tile_pool(name="pp", bufs=8) as pp, \
             tc.tile_pool(name="ptt", bufs=3) as ptt, \
             tc.tile_pool(name="osb", bufs=3) as osb, \
             tc.tile_pool(name="sps", bufs=2, space="PSUM") as sps:
            tri_sb = consts.tile([128, 128], F32)
            cos_sb = consts.tile([128, S], BF16)
            sin_sb = consts.tile([128, S], BF16)
            wq_sb = consts.tile([128, DTC, QH * 128], BF16)
            wk_sb = consts.tile([128, DTC, 128], BF16)
            wv_sb = consts.tile([128, DTC, 128], BF16)
            wo_sb = consts.tile([128, QH, D], BF16)

            qrot = [persist.tile([128, S], BF16, tag=f"qrot{h}", name=f"qrot{h}") for h in range(QH)]
            krot = persist.tile([128, S], BF16, tag="krot")
            vbf = persist.tile([128, S], BF16, tag="vbf")
            vnat = persist.tile([128, NT * 128], BF16, tag="vnat")
            vnat3 = vnat.rearrange("p (t f) -> p t f", t=NT)
            aout = [persist.tile([128, S], BF16, tag=f"aout{h}", name=f"aout{h}") for h in range(QH)]
            l_sb = stats.tile([128, QH * NT], F32, tag="l")
            linv_sb = stats.tile([128, QH * NT], F32, tag="linv")
            lpart = stats.tile([128, QH * NT * 2], F32, tag="lpart")

            units = [(g, h) for g in range(NT // 4) for h in range(QH)]

            def scores_gen(u):
                """QK chunks + mask + exp + normalize + P^T xbar for unit u.

                Yields after each score chunk so the caller can interleave
                other engine work; finishes by issuing the transpose DMAs
                into ptall and returning it via StopIteration.value.
                """
                g, h = units[u]
                for ii in range(4):
                    i = 4 * g + ii
                    W = (i + 1) * 128
                    p_i = pp.tile([128, S], BF16, tag="p", name=f"p{u}_{ii}")
                    col = h * NT + i
                    nch = (W + 1023) // 1024
                    for c in range(nch):
                        c0 = 1024 * c
                        ce = min(c0 + 1024, W)
                        s_ch = sps.tile([128, 1024], F32, tag="s")
                        for m0 in range(c0, ce, 512):
                            m1 = min(m0 + 512, ce)
                            nc.tensor.matmul(s_ch[:, m0 - c0:m1 - c0],
                                             qrot[h][:, i * 128:(i + 1) * 128],
                                             krot[:, m0:m1], start=True, stop=True)
                        if ce == W:   # diagonal block lives in this chunk
                            nc.vector.tensor_tensor(
                                out=s_ch[:, W - 128 - c0:W - c0],
                                in0=s_ch[:, W - 128 - c0:W - c0],
                                in1=tri_sb, op=ALU.add)
                        nc.scalar.activation(out=p_i[:, c0:ce], in_=s_ch[:, 0:ce - c0],
                                             func=AF.Exp, scale=SCALE,
                                             accum_out=lpart[:, col * 2 + c:col * 2 + c + 1])
                        yield
                    if nch > 1:
                        nc.vector.tensor_reduce(out=l_sb[:, col:col + 1],
                                                in_=lpart[:, col * 2:col * 2 + nch],
                                                axis=mybir.AxisListType.X, op=ALU.add)
                        nc.vector.reciprocal(out=linv_sb[:, col:col + 1],
                                             in_=l_sb[:, col:col + 1])
                    else:
                        nc.vector.reciprocal(out=linv_sb[:, col:col + 1],
                                             in_=lpart[:, col * 2:col * 2 + 1])
                    nc.vector.tensor_scalar_mul(p_i[:, 0:W], p_i[:, 0:W],
                                                linv_sb[:, col:col + 1])
                    if ii == 0:
                        ptall = ptt.tile([128, 4, NT, 128], BF16, tag="ptall",
                                         name=f"ptall{u}")
                    nc.sync.dma_start_transpose(
                        out=ptall[:, ii, 0:W // 128, :], in_=p_i[:, 0:W])
                return ptall

            def run_gen(gen):
                while True:
                    try:
                        next(gen)
                    except StopIteration as e:
                        return e.value

            # ---------------- QKV in quarters + early attention ------------
            hst_tiles = {}

            with tc.tile_pool(name="hsp", bufs=8) as hsp, \
                 tc.tile_pool(name="ropet", bufs=1) as ropet, \
                 tc.tile_pool(name="qkvps", bufs=1, space="PSUM") as qkvps:

                def load_hst(sh, j):
                    t = hsp.tile([128, 2, HALF], BF16, tag="hst", name=f"hst{sh}_{j}")
                    nc.sync.dma_start(
                        out=t, in_=hs3[:, 2 * j:2 * j + 2, sh * HALF:(sh + 1) * HALF])
                    hst_tiles[(sh, j)] = t
                    return t

                # DMA prologue: tiny mask, first weight chunks + hs tiles (so
                # the first matmuls start ~4us in), bulk weights, rest of hs
                # half 0, rope tables, wo (needed later).
                wq3 = wq_d.rearrange("p (t m) -> p t m", t=DTC)
                wk3 = wk_d.rearrange("p (t m) -> p t m", t=DTC)
                wv3 = wv_d.rearrange("p (t m) -> p t m", t=DTC)
                load_hst(0, 0)
                nc.sync.dma_start(out=wq_sb[:, 0:2, :], in_=wq3[:, 0:2, :])
                nc.sync.dma_start(out=wk_sb[:, 0:2, :], in_=wk3[:, 0:2, :])
                nc.sync.dma_start(out=wv_sb[:, 0:2, :], in_=wv3[:, 0:2, :])
                load_hst(0, 1)
                nc.sync.dma_start(out=tri_sb, in_=tri_d)
                nc.sync.dma_start(out=wq_sb[:, 2:DTC, :], in_=wq3[:, 2:DTC, :])
                nc.sync.dma_start(out=wk_sb[:, 2:DTC, :], in_=wk3[:, 2:DTC, :])
                nc.sync.dma_start(out=wv_sb[:, 2:DTC, :], in_=wv3[:, 2:DTC, :])
                for j in range(2, 8):
                    load_hst(0, j)
                nc.sync.dma_start(out=cos_sb, in_=cos_d)
                nc.sync.dma_start(out=sin_sb, in_=sin_d)
                nc.sync.dma_start(out=wo_sb, in_=wo_d.rearrange("p (h m) -> p h m", h=QH))

                early = []
                pending = []
                for qtr in range(4):
                    sh, qq = divmod(qtr, 2)
                    cols = slice(qtr * QTR, (qtr + 1) * QTR)
                    if qtr == 1:
                        for j in range(8):   # prefetch half 1 as slots free up
                            load_hst(1, j)
                    if qtr == 2:
                        pending = [scores_gen(0), scores_gen(1)]
                    if qtr == 3:
                        pending += [scores_gen(2), scores_gen(3), scores_gen(4)]
                    pq = [qkvps.tile([128, QTR], F32, tag=f"pq{m}", name=f"pq{m}") for m in range(QH)]
                    pk = qkvps.tile([128, QTR], F32, tag="pk")
                    pv = qkvps.tile([128, QTR], F32, tag="pv")
                    for j in range(DTC // 2):
                        hst = hst_tiles[(sh, j)]
                        for t2 in range(2):
                            dt = 2 * j + t2
                            st = dt == 0
                            sp = dt == DTC - 1
                            wlist = ([(wq_sb[:, dt, m * 128:(m + 1) * 128], pq[m]) for m in range(QH)]
                                     + [(wk_sb[:, dt, :], pk), (wv_sb[:, dt, :], pv)])
                            for w_ap, dst in wlist:
                                nc.tensor.matmul(dst, w_ap, hst[:, t2, qq * QTR:(qq + 1) * QTR],
                                                 start=st, stop=sp)
                        for _ in range(2 if qtr == 3 else 1):
                            if pending:
                                try:
                                    next(pending[0])
                                except StopIteration as e:
                                    early.append(e.value)
                                    pending.pop(0)
                    for m in range(QH):
                        _rope(nc, ropet, qrot[m], pq[m], cos_sb, sin_sb, cols, BF16, ALU)
                    _rope(nc, ropet, krot, pk, cos_sb, sin_sb, cols, BF16, ALU)
                    nc.scalar.copy(out=vbf[:, cols], in_=pv)
                    nc.sync.dma_start_transpose(
                        out=vnat3[:, 4 * qtr:4 * qtr + 4, :], in_=vbf[:, cols])


            # ---------------- attention tail + fused o_proj ----------------
            with tc.tile_pool(name="pvps", bufs=2, space="PSUM") as pvps, \
                 tc.tile_pool(name="pops", bufs=2, space="PSUM") as pops:

                deferred = []

                def oproj_tile(t, g):
                    o_sb = osb.tile([128, D], BF16, tag="osb")
                    for n in range(D // 512):
                        po = pops.tile([128, 512], F32, tag="po", name=f"po{t}_{n}")
                        for hh in range(QH):
                            nc.tensor.matmul(po, aout[hh][:, t * 128:(t + 1) * 128],
                                             wo_sb[:, hh, n * 512:(n + 1) * 512],
                                             start=(hh == 0), stop=(hh == QH - 1))
                        on_scalar = (n % 2 == 0) if g == 3 else (n == 0)
                        if on_scalar:
                            nc.scalar.copy(out=o_sb[:, n * 512:(n + 1) * 512], in_=po)
                        else:
                            nc.vector.tensor_copy(out=o_sb[:, n * 512:(n + 1) * 512], in_=po)
                    nc.sync.dma_start(out=out3[:, t, :], in_=o_sb)

                def stage_pv(u, ptall, pump):
                    """P@V accumulation + aout; o_proj split around unit seams."""
                    g, h = units[u]
                    jmax = 4 * g + 3
                    pv_ps = pvps.tile([128, 512], F32, tag="pv")
                    for j in range(jmax + 1):
                        ii_lo = max(0, j - 4 * g)
                        nc.tensor.matmul(pv_ps[:, ii_lo * 128:512],
                                         vnat[:, j * 128:(j + 1) * 128],
                                         ptall[:, ii_lo:4, j, :],
                                         start=(j == 0), stop=(j == jmax))
                        pump()
                        if deferred and j % 3 == 2:   # PE filler between PV steps
                            oproj_tile(*deferred.pop(0))
                    while deferred:
                        oproj_tile(*deferred.pop(0))
                    nc.scalar.copy(out=aout[h][:, g * 512:(g + 1) * 512],
                                   in_=pv_ps)
                    if h == QH - 1:   # both heads done: 1 tile now, 3 deferred
                        oproj_tile(4 * g, g)
                        deferred.extend([(4 * g + 1, g), (4 * g + 2, g),
                                         (4 * g + 3, g)])

                # Pump the next units' score generation between PV matmuls so
                # the in-order PE queue always has matmul work while softmax
                # (scalar/vector) of later units catches up.
                ptalls = dict(enumerate(early))
                live = {}
                nxt = len(early)
                for gen in pending:
                    live[nxt] = gen
                    nxt += 1
                for u in range(nxt, len(units)):
                    live[u] = scores_gen(u)

                def pump():
                    for u in sorted(live):
                        try:
                            next(live[u])
                        except StopIteration as e:
                            ptalls[u] = e.value
                            del live[u]
                        return

                for u in range(len(units)):
                    while u not in ptalls:   # finish its scores if still pending
                        pump()
                    stage_pv(u, ptalls.pop(u), pump)
                while deferred:
                    oproj_tile(*deferred.pop(0))

    nc.compile()
    return nc


def _pm(x):
    """[n*128, M] row-major -> partition-major [128, n*M]."""
    n = x.shape[0] // 128
    return np.ascontiguousarray(
        x.reshape(n, 128, x.shape[1]).transpose(1, 0, 2).reshape(128, -1))


def prep_in_maps(hidden_states, position_ids, Wq, Wk, Wv, Wo):
    import ml_dtypes
    BF = ml_dtypes.bfloat16
    hs = np.asarray(hidden_states, dtype=np.float32).reshape(S, D)
    hsT_pm = _pm(np.ascontiguousarray(hs.T)).astype(BF)             # [128, DTC*S]

    pos = np.asarray(position_ids).reshape(S).astype(np.float32)
    inv = (ROPE_BASE ** (-np.arange(0, HD, 2, dtype=np.float32) / HD))  # [64]
    ang = np.concatenate([pos[None, :] * inv[:, None]] * 2, axis=0)     # [128, S]
    cos_t = np.cos(ang).astype(BF)
    sin_t = np.sin(ang).astype(np.float32)
    sin_signed = np.concatenate([-sin_t[:64], sin_t[64:]], axis=0).astype(BF)

    q_idx = np.arange(128)[:, None]
    k_idx = np.arange(128)[None, :]
    tri = np.where(k_idx <= q_idx, 0.0, NEG).astype(np.float32)

    Wq = np.asarray(Wq, np.float32)
    Wk = np.asarray(Wk, np.float32)
    Wv = np.asarray(Wv, np.float32)
    Wo = np.asarray(Wo, np.float32)

    in_maps = []
    for c in range(NCORES):
        g = (c * QH) // (H // KV)          # kv head owned by this core
        wq_c = Wq[c * QH * 128:(c + 1) * QH * 128]      # [256, D]
        wk_c = Wk[g * 128:(g + 1) * 128]                # [128, D]
        wv_c = Wv[g * 128:(g + 1) * 128]                # [128, D]
        wo_c = Wo[:, c * QH * 128:(c + 1) * QH * 128]   # [D, 256]
        in_maps.append({
            "hs": hsT_pm,
            "wq": _pm(np.ascontiguousarray(wq_c.T)).astype(BF),
            "wk": _pm(np.ascontiguousarray(wk_c.T)).astype(BF),
            "wv": _pm(np.ascontiguousarray(wv_c.T)).astype(BF),
            "wo": _pm(np.ascontiguousarray(wo_c.T)).astype(BF),
            "cos": cos_t,
            "sin": sin_signed,
            "tri": tri,
        })
    return in_maps


def combine_outputs(results):
    total = np.zeros((S, D), np.float32)
    for r in results:
        o = np.asarray(r["out"], np.float32)
        total += o.reshape(128, NT, D).transpose(1, 0, 2).reshape(S, D)
    return total[None]


def kernel(hidden_states, attention_mask, position_ids, Wq, Wk, Wv, Wo):
    from concourse import bass_utils
    if "nc" not in _CACHE:
        _CACHE["nc"] = build_nc()
    nc = _CACHE["nc"]
    in_maps = prep_in_maps(hidden_states, position_ids, Wq, Wk, Wv, Wo)
    res = bass_utils.run_bass_kernel_spmd(nc, in_maps, core_ids=list(range(NCORES)))
    return combine_outputs(res.results)


# revision 40
# speedup vs baseline: 1.1027x; 1.0297x over previous
"""LlamaAttention (B=1, S=2048, D=2048, H=16, KV=4) on 8 TRN2 NeuronCores.

Tensor-parallel over heads: core c owns q-heads [2c, 2c+1] and kv-head c//2.
Each core computes partial = attn_out_c @ Wo[:, c-slice].T over the full
sequence; the all-reduce after o_proj happens on the host (sum of partials).

Layout strategy: everything on-chip lives feature-on-partitions ("transposed"):
  hsT [d, s], qT/kT/vT [hd, s], attn_outT [hd, s].  The host pre-transposes
hidden_states and weights into partition-major [128, N] bf16 arrays so every
DMA is contiguous; rope tables (bf16 cos / sign-adjusted sin) and the causal
diagonal mask block are precomputed on host.

Schedule (all matmuls bf16: fast weight load, half HBM):
 - DMA prologue orders the first hs tile + first weight chunks ahead of the
   bulk so the PE starts within a few microseconds.
 - QKV projects in four 512-column PSUM quarters (4 banks), leaving 4 banks
   for attention score chunks: the first five attention units' score/softmax
   chunks interleave into the back half of the projection matmul stream, so
   softmax runs under projection and the PE never idles at the phase seam.
 - P^T and natural-layout V come from xbar DMA transposes issued per tile
   right after normalization (no PE transposes, no PSUM drain copies); P@V
   reads the transposed tiles straight from SBUF through a strided AP.
 - Remaining units' score generation is pumped one chunk at a time between
   P@V accumulation steps, and each group's o_proj tiles are split around
   unit seams, so the in-order PE queue always has matmul work while
   exp (scalar) and normalization (vector) catch up.
 - softmax: exp with accum_out row sums (no running max: scores are O(6)
   sigma so fp32 exp cannot overflow); P normalized in sbuf by 1/l.
 - PSUM drains: rope uses one scalar cast then all-bf16 DVE ops (the
   rotate-half is a partition-shifted DVE copy); o_proj casts alternate
   scalar/vector.  Output partials are bf16, host all-reduces in f32.
"""
import math
import numpy as np

S = 2048
D = 2048
HD = 128
H = 16
KV = 4
NCORES = 8
NT = S // 128          # 16 sequence tiles
DTC = D // 128         # 16 feature chunks
QH = H // NCORES       # 2 q-heads per core
ROPE_BASE = 10000.0
SCALE = 1.0 / math.sqrt(HD)
NEG = -1.0e9

_CACHE = {}


def _rope(nc, pool, dst, src_ps, cos_sb, sin_sb, cols, BF16, ALU):
    """dst[:, cols] = src*cos + rotate_half(src)*sin  (src: psum [128, w]).

    One scalar drain (psum->bf16 sbuf), then all-bf16 SBUF vector ops which
    run in the DVE's fast 2x/4x modes; the rotate-half is a partition-shifted
    DVE copy (legal on TRN2).
    """
    w = cols.stop - cols.start
    raw = pool.tile([128, w], BF16, tag="roperaw")
    rot = pool.tile([128, w], BF16, tag="roperot")
    t1 = pool.tile([128, w], BF16, tag="ropet1")
    nc.scalar.copy(out=raw, in_=src_ps)
    nc.vector.tensor_copy(out=rot[0:64, :], in_=raw[64:128, :])
    nc.vector.tensor_copy(out=rot[64:128, :], in_=raw[0:64, :])
    nc.vector.tensor_tensor(out=t1, in0=raw, in1=cos_sb[:, cols], op=ALU.mult)
    nc.vector.tensor_tensor(out=rot, in0=rot, in1=sin_sb[:, cols], op=ALU.mult)
    nc.vector.tensor_tensor(out=dst[:, cols], in0=t1, in1=rot, op=ALU.add)


def build_nc():
    import concourse.bacc as bacc
    import concourse.tile as tile
    from concourse import mybir

    F32 = mybir.dt.float32
    BF16 = mybir.dt.bfloat16
    AF = mybir.ActivationFunctionType
    ALU = mybir.AluOpType

    nc = bacc.Bacc("TRN2", target_bir_lowering=False, debug=False)
    hs_d = nc.dram_tensor("hs", [128, DTC * S], BF16, kind="ExternalInput").ap()
    wq_d = nc.dram_tensor("wq", [128, DTC * QH * 128], BF16, kind="ExternalInput").ap()
    wk_d = nc.dram_tensor("wk", [128, DTC * 128], BF16, kind="ExternalInput").ap()
    wv_d = nc.dram_tensor("wv", [128, DTC * 128], BF16, kind="ExternalInput").ap()
    wo_d = nc.dram_tensor("wo", [128, QH * D], BF16, kind="ExternalInput").ap()
    cos_d = nc.dram_tensor("cos", [128, S], BF16, kind="ExternalInput").ap()
    sin_d = nc.dram_tensor("sin", [128, S], BF16, kind="ExternalInput").ap()
    tri_d = nc.dram_tensor("tri", [128, 128], F32, kind="ExternalInput").ap()
    out_d = nc.dram_tensor("out", [128, NT * D], BF16, kind="ExternalOutput").ap()

    hs3 = hs_d.rearrange("p (t s) -> p t s", t=DTC)
    out3 = out_d.rearrange("p (t d) -> p t d", t=NT)

    HALF = S // 2
    QTR = S // 4

    with tile.TileContext(nc) as tc:
        with tc.tile_pool(name="consts", bufs=1) as consts, \
             tc.tile_pool(name="persist", bufs=1) as persist, \
             tc.tile_pool(name="stats", bufs=1) as stats, \
             tc.tile_pool(name="pp", bufs=8) as pp, \
             tc.tile_pool(name="ptt", bufs=3) as ptt, \
             tc.tile_pool(name="osb", bufs=3) as osb, \
             tc.tile_pool(name="sps", bufs=2, space="PSUM") as sps:
            tri_sb = consts.tile([128, 128], F32)
            cos_sb = consts.tile([128, S], BF16)
            sin_sb = consts.tile([128, S], BF16)
            wq_sb = consts.tile([128, DTC, QH * 128], BF16)
            wk_sb = consts.tile([128, DTC, 128], BF16)
            wv_sb = consts.tile([128, DTC, 128], BF16)
            wo_sb = consts.tile([128, QH, D], BF16)

            qrot = [persist.tile([128, S], BF16, tag=f"qrot{h}", name=f"qrot{h}") for h in range(QH)]
            krot = persist.tile([128, S], BF16, tag="krot")
            vbf = persist.tile([128, S], BF16, tag="vbf")
            vnat = persist.tile([128, NT * 128], BF16, tag="vnat")
            vnat3 = vnat.rearrange("p (t f) -> p t f", t=NT)
            aout = [persist.tile([128, S], BF16, tag=f"aout{h}", name=f"aout{h}") for h in range(QH)]
            l_sb = stats.tile([128, QH * NT], F32, tag="l")
            linv_sb = stats.tile([128, QH * NT], F32, tag="linv")
            lpart = stats.tile([128, QH * NT * 2], F32, tag="lpart")

            units = [(g, h) for g in range(NT // 4) for h in range(QH)]

            def scores_gen(u):
                """QK chunks + mask + exp + normalize + P^T xbar for unit u.

                Yields after each score chunk so the caller can interleave
                other engine work; finishes by issuing the transpose DMAs
                into ptall and returning it via StopIteration.value.
                """
                g, h = units[u]
                for ii in range(4):
                    i = 4 * g + ii
                    W = (i + 1) * 128
                    p_i = pp.tile([128, S], BF16, tag="p", name=f"p{u}_{ii}")
                    col = h * NT + i
                    nch = (W + 1023) // 1024
                    for c in range(nch):
                        c0 = 1024 * c
                        ce = min(c0 + 1024, W)
                        s_ch = sps.tile([128, 1024], F32, tag="s")
                        for m0 in range(c0, ce, 512):
                            m1 = min(m0 + 512, ce)
                            nc.tensor.matmul(s_ch[:, m0 - c0:m1 - c0],
                                             qrot[h][:, i * 128:(i + 1) * 128],
                                             krot[:, m0:m1], start=True, stop=True)
                        if ce == W:   # diagonal block lives in this chunk
                            nc.vector.tensor_tensor(
                                out=s_ch[:, W - 128 - c0:W - c0],
                                in0=s_ch[:, W - 128 - c0:W - c0],
                                in1=tri_sb, op=ALU.add)
                        nc.scalar.activation(out=p_i[:, c0:ce], in_=s_ch[:, 0:ce - c0],
                                             func=AF.Exp, scale=SCALE,
                                             accum_out=lpart[:, col * 2 + c:col * 2 + c + 1])
                        yield
                    if nch > 1:
                        nc.vector.tensor_reduce(out=l_sb[:, col:col + 1],
                                                in_=lpart[:, col * 2:col * 2 + nch],
                                                axis=mybir.AxisListType.X, op=ALU.add)
                        nc.vector.reciprocal(out=linv_sb[:, col:col + 1],
                                             in_=l_sb[:, col:col + 1])
                    else:
                        nc.vector.reciprocal(out=linv_sb[:, col:col + 1],
                                             in_=lpart[:, col * 2:col * 2 + 1])
                    nc.vector.tensor_scalar_mul(p_i[:, 0:W], p_i[:, 0:W],
                                                linv_sb[:, col:col + 1])
                    if ii == 0:
                        ptall = ptt.tile([128, 4, NT, 128], BF16, tag="ptall",
                                         name=f"ptall{u}")
                    nc.sync.dma_start_transpose(
                        out=ptall[:, ii, 0:W // 128, :], in_=p_i[:, 0:W])
                return ptall

            def run_gen(gen):
                while True:
                    try:
                        next(gen)
                    except StopIteration as e:
                        return e.value

            # ---------------- QKV in quarters + early attention ------------
            hst_tiles = {}

            with tc.tile_pool(name="hsp", bufs=8) as hsp, \
                 tc.tile_pool(name="ropet", bufs=1) as ropet, \
                 tc.tile_pool(name="qkvps", bufs=1, space="PSUM") as qkvps:

                def load_hst(sh, j):
                    t = hsp.tile([128, 2, HALF], BF16, tag="hst", name=f"hst{sh}_{j}")
                    nc.sync.dma_start(
                        out=t, in_=hs3[:, 2 * j:2 * j + 2, sh * HALF:(sh + 1) * HALF])
                    hst_tiles[(sh, j)] = t
                    return t

                # DMA prologue: tiny mask, first weight chunks + hs tiles (so
                # the first matmuls start ~4us in), bulk weights, rest of hs
                # half 0, rope tables, wo (needed later).
                wq3 = wq_d.rearrange("p (t m) -> p t m", t=DTC)
                wk3 = wk_d.rearrange("p (t m) -> p t m", t=DTC)
                wv3 = wv_d.rearrange("p (t m) -> p t m", t=DTC)
                load_hst(0, 0)
                nc.sync.dma_start(out=wq_sb[:, 0:2, :], in_=wq3[:, 0:2, :])
                nc.sync.dma_start(out=wk_sb[:, 0:2, :], in_=wk3[:, 0:2, :])
                nc.sync.dma_start(out=wv_sb[:, 0:2, :], in_=wv3[:, 0:2, :])
                load_hst(0, 1)
                nc.sync.dma_start(out=tri_sb, in_=tri_d)
                nc.sync.dma_start(out=wq_sb[:, 2:DTC, :], in_=wq3[:, 2:DTC, :])
                nc.sync.dma_start(out=wk_sb[:, 2:DTC, :], in_=wk3[:, 2:DTC, :])
                nc.sync.dma_start(out=wv_sb[:, 2:DTC, :], in_=wv3[:, 2:DTC, :])
                for j in range(2, 8):
                    load_hst(0, j)
                nc.sync.dma_start(out=cos_sb, in_=cos_d)
                nc.sync.dma_start(out=sin_sb, in_=sin_d)
                nc.sync.dma_start(out=wo_sb, in_=wo_d.rearrange("p (h m) -> p h m", h=QH))

                early = []
                pending = []
                for qtr in range(4):
                    sh, qq = divmod(qtr, 2)
                    cols = slice(qtr * QTR, (qtr + 1) * QTR)
                    if qtr == 1:
                        for j in range(8):   # prefetch half 1 as slots free up
                            load_hst(1, j)
                    if qtr == 2:
                        pending = [scores_gen(0), scores_gen(1)]
                    if qtr == 3:
                        pending += [scores_gen(2), scores_gen(3), scores_gen(4)]
                    pq = [qkvps.tile([128, QTR], F32, tag=f"pq{m}", name=f"pq{m}") for m in range(QH)]
                    pk = qkvps.tile([128, QTR], F32, tag="pk")
                    pv = qkvps.tile([128, QTR], F32, tag="pv")
                    for j in range(DTC // 2):
                        hst = hst_tiles[(sh, j)]
                        for t2 in range(2):
                            dt = 2 * j + t2
                            st = dt == 0
                            sp = dt == DTC - 1
                            wlist = ([(wq_sb[:, dt, m * 128:(m + 1) * 128], pq[m]) for m in range(QH)]
                                     + [(wk_sb[:, dt, :], pk), (wv_sb[:, dt, :], pv)])
                            for w_ap, dst in wlist:
                                nc.tensor.matmul(dst, w_ap, hst[:, t2, qq * QTR:(qq + 1) * QTR],
                                                 start=st, stop=sp)
                        for _ in range(2 if qtr == 3 else 1):
                            if pending:
                                try:
                                    next(pending[0])
                                except StopIteration as e:
                                    early.append(e.value)
                                    pending.pop(0)
                    for m in range(QH):
                        _rope(nc, ropet, qrot[m], pq[m], cos_sb, sin_sb, cols, BF16, ALU)
                    _rope(nc, ropet, krot, pk, cos_sb, sin_sb, cols, BF16, ALU)
                    nc.scalar.copy(out=vbf[:, cols], in_=pv)
                    nc.sync.dma_start_transpose(
                        out=vnat3[:, 4 * qtr:4 * qtr + 4, :], in_=vbf[:, cols])


                while pending:
                    early.append(run_gen(pending.pop(0)))

            # ---------------- attention tail + fused o_proj ----------------
            with tc.tile_pool(name="pvps", bufs=2, space="PSUM") as pvps, \
                 tc.tile_pool(name="pops", bufs=2, space="PSUM") as pops:

                deferred = []

                def oproj_tile(t, g):
                    o_sb = osb.tile([128, D], BF16, tag="osb")
                    for n in range(D // 512):
                        po = pops.tile([128, 512], F32, tag="po", name=f"po{t}_{n}")
                        for hh in range(QH):
                            nc.tensor.matmul(po, aout[hh][:, t * 128:(t + 1) * 128],
                                             wo_sb[:, hh, n * 512:(n + 1) * 512],
                                             start=(hh == 0), stop=(hh == QH - 1))
                        on_scalar = (n % 2 == 0) if g == 3 else (n == 0)
                        if on_scalar:
                            nc.scalar.copy(out=o_sb[:, n * 512:(n + 1) * 512], in_=po)
                        else:
                            nc.vector.tensor_copy(out=o_sb[:, n * 512:(n + 1) * 512], in_=po)
                    nc.sync.dma_start(out=out3[:, t, :], in_=o_sb)

                def stage_pv(u, ptall, pump):
                    """P@V accumulation + aout; o_proj split around unit seams."""
                    g, h = units[u]
                    jmax = 4 * g + 3
                    pv_ps = pvps.tile([128, 512], F32, tag="pv")
                    for j in range(jmax + 1):
                        ii_lo = max(0, j - 4 * g)
                        nc.tensor.matmul(pv_ps[:, ii_lo * 128:512],
                                         vnat[:, j * 128:(j + 1) * 128],
                                         ptall[:, ii_lo:4, j, :],
                                         start=(j == 0), stop=(j == jmax))
                        pump()
                        if deferred and j % 3 == 2:   # PE filler between PV steps
                            oproj_tile(*deferred.pop(0))
                    while deferred:
                        oproj_tile(*deferred.pop(0))
                    nc.scalar.copy(out=aout[h][:, g * 512:(g + 1) * 512],
                                   in_=pv_ps)
                    if h == QH - 1:   # both heads done: 1 tile now, 3 deferred
                        oproj_tile(4 * g, g)
                        deferred.extend([(4 * g + 1, g), (4 * g + 2, g),
                                         (4 * g + 3, g)])

                # Pump the next units' score generation between PV matmuls so
                # the in-order PE queue always has matmul work while softmax
                # (scalar/vector) of later units catches up.
                ptalls = dict(enumerate(early))
                live = {u: scores_gen(u) for u in range(len(early), len(units))}

                def pump():
                    for u in sorted(live):
                        try:
                            next(live[u])
                        except StopIteration as e:
                            ptalls[u] = e.value
                            del live[u]
                        return

                for u in range(len(units)):
                    while u not in ptalls:   # finish its scores if still pending
                        pump()
                    stage_pv(u, ptalls.pop(u), pump)
                while deferred:
                    oproj_tile(*deferred.pop(0))

    nc.compile()
    return nc


def _pm(x):
    """[n*128, M] row-major -> partition-major [128, n*M]."""
    n = x.shape[0] // 128
    return np.ascontiguousarray(
        x.reshape(n, 128, x.shape[1]).transpose(1, 0, 2).reshape(128, -1))


def prep_in_maps(hidden_states, position_ids, Wq, Wk, Wv, Wo):
    import ml_dtypes
    BF = ml_dtypes.bfloat16
    hs = np.asarray(hidden_states, dtype=np.float32).reshape(S, D)
    hsT_pm = _pm(np.ascontiguousarray(hs.T)).astype(BF)             # [128, DTC*S]

    pos = np.asarray(position_ids).reshape(S).astype(np.float32)
    inv = (ROPE_BASE ** (-np.arange(0, HD, 2, dtype=np.float32) / HD))  # [64]
    ang = np.concatenate([pos[None, :] * inv[:, None]] * 2, axis=0)     # [128, S]
    cos_t = np.cos(ang).astype(BF)
    sin_t = np.sin(ang).astype(np.float32)
    sin_signed = np.concatenate([-sin_t[:64], sin_t[64:]], axis=0).astype(BF)

    q_idx = np.arange(128)[:, None]
    k_idx = np.arange(128)[None, :]
    tri = np.where(k_idx <= q_idx, 0.0, NEG).astype(np.float32)

    Wq = np.asarray(Wq, np.float32)
    Wk = np.asarray(Wk, np.float32)
    Wv = np.asarray(Wv, np.float32)
    Wo = np.asarray(Wo, np.float32)

    in_maps = []
    for c in range(NCORES):
        g = (c * QH) // (H // KV)          # kv head owned by this core
        wq_c = Wq[c * QH * 128:(c + 1) * QH * 128]      # [256, D]
        wk_c = Wk[g * 128:(g + 1) * 128]                # [128, D]
        wv_c = Wv[g * 128:(g + 1) * 128]                # [128, D]
        wo_c = Wo[:, c * QH * 128:(c + 1) * QH * 128]   # [D, 256]
        in_maps.append({
            "hs": hsT_pm,
            "wq": _pm(np.ascontiguousarray(wq_c.T)).astype(BF),
            "wk": _pm(np.ascontiguousarray(wk_c.T)).astype(BF),
            "wv": _pm(np.ascontiguousarray(wv_c.T)).astype(BF),
            "wo": _pm(np.ascontiguousarray(wo_c.T)).astype(BF),
            "cos": cos_t,
            "sin": sin_signed,
            "tri": tri,
        })
    return in_maps


def combine_outputs(results):
    total = np.zeros((S, D), np.float32)
    for r in results:
        o = np.asarray(r["out"], np.float32)
        total += o.reshape(128, NT, D).transpose(1, 0, 2).reshape(S, D)
    return total[None]


def kernel(hidden_states, attention_mask, position_ids, Wq, Wk, Wv, Wo):
    from concourse import bass_utils
    if "nc" not in _CACHE:
        _CACHE["nc"] = build_nc()
    nc = _CACHE["nc"]
    in_maps = prep_in_maps(hidden_states, position_ids, Wq, Wk, Wv, Wo)
    res = bass_utils.run_bass_kernel_spmd(nc, in_maps, core_ids=list(range(NCORES)))
    return combine_outputs(res.results)
